# revision 23
# baseline (speedup 1.0000x reference)
"""Trainium2 Bass kernel for nn_DecoderLayer (self-attn + cross-attn + FFN).

Sharding: data-parallel over batch, 4 batch elements per core x 8 cores.
Each core runs an identical (SPMD) Tile program on its own shard; no
collectives.

v2: fp8(e4m3) DoubleRow matmuls for SA Q/K/V/scores/out-proj and
CA Q/scores/ctx/out-proj (CA K/V were already fp8-DR); FFN and SA ctx
stay bf16. Softmax skips the max-subtraction (scores are tiny: |s|*scale
<~1.5). PSUM evacuations alternate DVE/ACT; gpsimd takes SBUF->SBUF
copies. Measured L2 rel err ~1.1e-2 on host sim (gate 2e-2).

Layouts (per core, T = 4*128 = 512 decoder tokens, LE = 512 enc tokens):
  xT8     [2, 128, 2, T]  fp8  dec inputs feature-major in DoubleRow
                               layout: [c, p, i, t] = x[c*256+i*128+p, t]
  x0      [T, D]  f32          dec inputs token-major (residual + sa_bo)
  encT8   [BPC, 2, 128, 2, LE] fp8 enc outputs feature-major DR layout
  maskneg [128, T] f32         -1e9 where masked, [q, e*128+k]
DR matmul operands are [128, 2, N] fp8 (contraction pairs on the middle
axis); weight tensors are host-packed into that layout (pack8).
"""

import contextlib
import os
import sys

for _p in ('/opt/trn_rl_repo', '/root/.axon_site/_ro/trn_rl_repo'):
    if os.path.isdir(_p) and _p not in sys.path:
        sys.path.append(_p)

import numpy as np
import ml_dtypes

import concourse.bass as bass
import concourse.tile as tile
import concourse.mybir as mybir
from concourse import bacc
from concourse.bass_utils import run_bass_kernel_spmd
from concourse.masks import make_identity

F32 = mybir.dt.float32
BF16 = mybir.dt.bfloat16
FP8 = mybir.dt.float8e4
DR = mybir.MatmulPerfMode.DoubleRow
AF = mybir.ActivationFunctionType
ALU = mybir.AluOpType
AX = mybir.AxisListType

B, LD, LE, D, H, R = 32, 128, 512, 512, 8, 4
DH = D * H            # 4096
DF = D * R            # 2048
NCORES = 8
BPC = B // NCORES     # 4 batch elements per core
T = BPC * LD          # 512 decoder tokens per core
KC = D // 128         # 4 contraction chunks of 128
SCALE = float(1.0 / np.sqrt(D))

_CACHE = {}


class _Eng:
    """Alternating DVE/ACT picker for PSUM->SBUF evacuation."""

    def __init__(self, nc, pat="01"):
        self.nc = nc
        self.pat = pat
        self.i = 0

    def copy(self, out, in_, bias=None):
        nc = self.nc
        self.i = (self.i + 1) % len(self.pat)
        if self.pat[self.i] == "0":
            if bias is None:
                nc.vector.tensor_copy(out=out, in_=in_)
            else:
                nc.vector.tensor_scalar_add(out, in_, bias)
        else:
            if bias is None:
                nc.scalar.copy(out, in_)
            else:
                nc.scalar.activation(out=out, in_=in_, func=AF.Identity, bias=bias)


_POOLSPEC = [
    ("const", 1, "SBUF"), ("aring", 72, "SBUF"), ("wp", 20, "SBUF"),
    ("wop", 3, "SBUF"), ("encp", 8, "SBUF"),
    ("xfp", 6, "SBUF"), ("accp", 6, "SBUF"), ("xtp", 8, "SBUF"),
    ("htp", 16, "SBUF"), ("ctp", 12, "SBUF"), ("pp", 8, "SBUF"),
    ("ptp", 16, "SBUF"), ("stp", 24, "SBUF"), ("bnp", 4, "SBUF"),
    ("psP", 2, "PSUM"), ("psS", 2, "PSUM"), ("psC", 2, "PSUM"),
    ("psT", 2, "PSUM"),
]


def _build(loop_n=1):
    nc = bacc.Bacc("TRN2", target_bir_lowering=False, debug=False,
                   num_devices=NCORES)

    def din(name, shape, dt):
        return nc.dram_tensor(name, shape, dt, kind="ExternalInput").ap()

    xT8_d = din("xT8", [2, 128, 2, T], FP8)
    x0_d = din("x0", [T, D], F32)
    encT_d = din("encT8", [BPC, 2, 128, 2, LE], FP8)
    mask_d = din("maskneg", [LD, T], F32)

    w_d = {}
    for pre in ("sa", "ca"):
        for nm in "qkv" if pre == "sa" else "qkv":
            w_d[f"{pre}_{nm}8"] = din(f"w_{pre}{nm}8", [2, 128, 2, DH], FP8)
        w_d[f"{pre}_o8"] = din(f"w_{pre}o8", [H, 2, 128, 2, D], FP8)
    w_d["ff1"] = din("w_ff1", [D, DF], BF16)
    w_d["ff2"] = din("w_ff2", [DF, D], BF16)

    bp_d = {k: din(f"bp_{k}", [128, DH // 128], F32)
            for k in ("saq", "sak", "sav", "caq", "cak", "cav")}

    out_d = nc.dram_tensor("out", [T, D], F32, kind="ExternalOutput").ap()

    with tile.TileContext(nc) as tc:
        with contextlib.ExitStack() as _st:
            pools = {}
            for _nm, _bufs, _sp in _POOLSPEC:
                pools[_nm] = _st.enter_context(
                    tc.tile_pool(name=_nm, bufs=_bufs, space=_sp))
            if loop_n > 1:
                _st.enter_context(tc.For_i(0, loop_n, 1))
            _emit(nc, tc, pools, xT8_d, x0_d, encT_d, mask_d,
                  w_d, bp_d, out_d)
    nc.compile()
    return nc


def _emit(nc, tc, pools, xT8_d, x0_d, encT_d, mask_d, w_d, bp_d, out_d):
    cpool, ar, encp, xfp = pools["const"], pools["aring"], pools["encp"], pools["xfp"]
    wpool = pools["wp"]
    accp, xtp, htp, ctp = pools["accp"], pools["xtp"], pools["htp"], pools["ctp"]
    ppool, ptp, stp, bnp = pools["pp"], pools["ptp"], pools["stp"], pools["bnp"]
    psP, psS, psC, psT = pools["psP"], pools["psS"], pools["psC"], pools["psT"]

    eng = _Eng(nc)

    # ---------------- constants ----------------
    ident_bf = cpool.tile([128, 128], BF16, tag="idb", name="idb")
    make_identity(nc, ident_bf)
    ident_f32 = cpool.tile([128, 128], F32, tag="idf", name="idf")
    make_identity(nc, ident_f32)
    eps_t = cpool.tile([128, 1], F32, tag="eps", name="eps")
    nc.vector.memset(eps_t, 1e-5)

    # ---------------- activations in ----------------
    xT8 = []
    for c in range(2):
        t = xtp.tile([128, 2, T], FP8, tag="xt8", name="xt8")
        nc.sync.dma_start(out=t, in_=xT8_d[c])
        xT8.append(t)
    mask_t = cpool.tile([128, T], F32, tag="mask", name="mask")
    nc.sync.dma_start(out=mask_t, in_=mask_d)
    bp = {}
    for k, d in bp_d.items():
        t = cpool.tile([128, DH // 128], F32, tag=f"bp_{k}", name=f"bp_{k}")
        nc.sync.dma_start(out=t, in_=d)
        bp[k] = t

    def load_w8(key, h, dmae=None):
        """[2] tiles [128, 2, 512] fp8 from dram [2, 128, 2, DH]."""
        ts = []
        for c in range(2):
            t = wpool.tile([128, 2, 512], FP8, tag="w8", name="w8")
            (dmae or nc.sync).dma_start(
                out=t, in_=w_d[key][c, :, :, h * 512:(h + 1) * 512])
            ts.append(t)
        return ts

    def load_wo8(key, h):
        """[2] tiles [128, 2, 512] fp8 from dram [H, 2, 128, 2, D]."""
        ts = []
        for c in range(2):
            t = wpool.tile([128, 2, 512], FP8, tag="w8", name="w8")
            nc.sync.dma_start(out=t, in_=w_d[key][h, c])
            ts.append(t)
        return ts

    def proj_dr8(w2, rhs2, bias_key, h, width=T):
        """DR projection -> 2 tiles [128, 2, width] fp8 (DR layout)."""
        outs = [ar.tile([128, 2, width], FP8, tag="a", name="a")
                for _ in range(2)]
        for dco in range(KC):
            ps = psP.tile([128, width], F32, tag="pp", name="pp")
            for c in range(2):
                nc.tensor.matmul(ps, w2[c][:, :, dco * 128:(dco + 1) * 128],
                                 rhs2[c], start=(c == 0), stop=(c == 1),
                                 perf_mode=DR)
            bcol = bp[bias_key][:, h * 4 + dco:h * 4 + dco + 1]
            eng.copy(outs[dco // 2][:, dco % 2, :], ps, bias=bcol)
        return outs

    def proj_bf(w2, rhs2, bias_key, h, width=T):
        """DR projection -> KC tiles [128, width] bf16 (plain layout)."""
        outs = []
        for dco in range(KC):
            ps = psP.tile([128, width], F32, tag="pp", name="pp")
            for c in range(2):
                nc.tensor.matmul(ps, w2[c][:, :, dco * 128:(dco + 1) * 128],
                                 rhs2[c], start=(c == 0), stop=(c == 1),
                                 perf_mode=DR)
            t = ar.tile([128, width], BF16, tag="a", name="a")
            bcol = bp[bias_key][:, h * 4 + dco:h * 4 + dco + 1]
            eng.copy(t, ps, bias=bcol)
            outs.append(t)
        return outs

    def softmax_np(ps_s, width, p_tag):
        """exp(scale*s) with row-sum accum; no max subtraction.

        Returns (unnormalized P tile, 1/rowsum [128,1]); the 1/rowsum is
        folded into the residual update after the output projection
        (exact given zero V/O biases, asserted host-side).
        """
        p_t = ppool.tile([128, width], BF16, tag=p_tag, name=p_tag)
        rs = stp.tile([128, 1], F32, tag="st", name="st")
        nc.scalar.activation(out=p_t, in_=ps_s, func=AF.Exp,
                             scale=SCALE, accum_out=rs)
        r = stp.tile([128, 1], F32, tag="st", name="st")
        nc.vector.reciprocal(r, rs)
        return p_t, r

    def layer_norm(acc, out_tag):
        """returns normed f32 tile (gamma=1/beta=0 asserted host-side)."""
        bn = bnp.tile([128, 6], F32, tag="bn", name="bn")
        nc.vector.bn_stats(out=bn, in_=acc)
        mv = bnp.tile([128, 2], F32, tag="mv", name="mv")
        nc.vector.bn_aggr(out=mv, in_=bn)
        std = stp.tile([128, 1], F32, tag="st", name="st")
        nc.scalar.activation(out=std, in_=mv[:, 1:2], func=AF.Sqrt,
                             bias=eps_t)
        rstd = stp.tile([128, 1], F32, tag="st", name="st")
        nc.vector.reciprocal(rstd, std)
        nb = stp.tile([128, 1], F32, tag="st", name="st")
        nc.vector.tensor_scalar(out=nb, in0=mv[:, 0:1], scalar1=rstd,
                                scalar2=-1.0, op0=ALU.mult, op1=ALU.mult)
        xn = xfp.tile([128, D], F32, tag=out_tag, name=out_tag)
        nc.scalar.activation(out=xn, in_=acc, func=AF.Identity,
                             scale=rstd, bias=nb)
        return xn

    # ================= self attention =================
    acc_sa = [None] * BPC
    x0 = []

    def sa_proj(h):
        dmae = nc.gpsimd if h == 0 else None
        wq = load_w8("sa_q8", h, dmae)
        wk = load_w8("sa_k8", h, dmae)
        wv = load_w8("sa_v8", h)
        wo = load_wo8("sa_o8", h)
        qth = proj_bf(wq, xT8, "saq", h)
        kth = proj_bf(wk, xT8, "sak", h)
        vh = []
        for e in range(BPC):
            ps = psP.tile([128, 512], F32, tag="pp", name="pp")
            for c in range(2):
                nc.tensor.matmul(ps, xT8[c][:, :, e * 128:(e + 1) * 128],
                                 wv[c], start=(c == 0), stop=(c == 1),
                                 perf_mode=DR)
            t = ar.tile([128, 512], BF16, tag="a", name="a")
            eng.copy(t, ps)
            vh.append(t)
        return qth, kth, vh, wo

    def sa_scores(h, e, proj):
        qth, kth, vh, wo = proj
        sl = slice(e * 128, (e + 1) * 128)
        ps_s = psS.tile([128, 512], F32, tag="ps", name="ps")
        ss = ps_s[:, 0:128]
        for dc in range(KC):
            nc.tensor.matmul(ss, qth[dc][:, sl], kth[dc][:, sl],
                             start=(dc == 0), stop=(dc == KC - 1))
        nc.vector.tensor_add(ss, ss, mask_t[:, sl])
        return softmax_np(ss, 128, "psa")

    def sa_tail(h, e, proj, p_r):
        _, _, vh, wo = proj
        p_t, r = p_r
        pt_t = ptp.tile([128, 128], BF16, tag="pts", name="pts")
        nc.sync.dma_start_transpose(out=pt_t, in_=p_t)
        ps_c = psC.tile([128, 512], F32, tag="pc", name="pc")
        for dc in range(KC):
            nc.tensor.matmul(ps_c[:, dc * 128:(dc + 1) * 128],
                             vh[e][:, dc * 128:(dc + 1) * 128], pt_t,
                             start=True, stop=True)
        ct8 = [ptp.tile([128, 2, 128], FP8, tag="ct8", name="ct8")
               for _ in range(2)]
        for c in range(2):
            eng.copy(ct8[c].rearrange("p i j -> p (i j)"),
                     ps_c[:, c * 256:(c + 1) * 256])
        ps_o = psP.tile([128, 512], F32, tag="pp", name="pp")
        for c in range(2):
            nc.tensor.matmul(ps_o, ct8[c], wo[c], start=(c == 0),
                             stop=(c == 1), perf_mode=DR)
        if h == 0:
            t = xfp.tile([128, D], F32, tag="x", name="x")
            nc.sync.dma_start(out=t, in_=x0_d[e * 128:(e + 1) * 128, :])
            x0.append(t)
            acc_sa[e] = accp.tile([128, D], F32, tag="acc", name="acc")
            nc.vector.scalar_tensor_tensor(out=acc_sa[e], in0=ps_o, scalar=r,
                                           in1=x0[e], op0=ALU.mult,
                                           op1=ALU.add)
        else:
            nc.vector.scalar_tensor_tensor(out=acc_sa[e], in0=ps_o, scalar=r,
                                           in1=acc_sa[e], op0=ALU.mult,
                                           op1=ALU.add)

    encT = [[None] * 2 for _ in range(BPC)]

    def load_enc():
        for e in range(BPC):
            for c in range(2):
                t = encp.tile([128, 2, LE], FP8, tag="enc", name="enc")
                nc.sync.dma_start(out=t, in_=encT_d[e, c])
                encT[e][c] = t

    pend = []
    for h in range(H):
        proj = sa_proj(h)
        if h == 6:
            load_enc()
        for e in range(BPC):
            p_t = sa_scores(h, e, proj)
            pend.append((h, e, proj, p_t))
            if len(pend) > 4:
                sa_tail(*pend.pop(0))
    for u in pend:
        sa_tail(*u)

    # ================= cross attention =================
    acc_ca = [None] * BPC

    def ca_proj(h):
        wk = load_w8("ca_k8", h)
        wv = load_w8("ca_v8", h)
        wo = load_wo8("ca_o8", h)
        qt8 = proj_dr8(load_w8("ca_q8", h), x1t8, "caq", h)
        return wk, wv, wo, qt8

    def ca_kv(h, e, wk, wv):
        """per-elem K (fp8 DR layout, for DR scores) and V (bf16 token-major)."""
        kt8e = [ar.tile([128, 2, LE], FP8, tag="a", name="a") for _ in range(2)]
        for mc in range(KC):
            ps = psP.tile([128, LE], F32, tag="pp", name="pp")
            for c in range(2):
                nc.tensor.matmul(ps, wk[c][:, :, mc * 128:(mc + 1) * 128],
                                 encT[e][c], start=(c == 0), stop=(c == 1),
                                 perf_mode=DR)
            eng.copy(kt8e[mc // 2][:, mc % 2, :], ps,
                     bias=bp["cak"][:, h * 4 + mc:h * 4 + mc + 1])
        ve = []
        for tc_ in range(KC):
            ps = psP.tile([128, 512], F32, tag="pp", name="pp")
            for c in range(2):
                nc.tensor.matmul(ps, encT[e][c][:, :, tc_ * 128:(tc_ + 1) * 128],
                                 wv[c], start=(c == 0), stop=(c == 1),
                                 perf_mode=DR)
            t = ar.tile([128, 512], BF16, tag="a", name="a")
            eng.copy(t, ps)
            ve.append(t)
        return kt8e, ve

    def ca_scores(h, e, proj, kv=None):
        wk, wv, wo, qt8 = proj
        kt8e, ve = kv if kv is not None else ca_kv(h, e, wk, wv)
        sl = slice(e * 128, (e + 1) * 128)
        ps_s = psS.tile([128, LE], F32, tag="ps", name="ps")
        for c in range(2):
            nc.tensor.matmul(ps_s, qt8[c][:, :, sl], kt8e[c],
                             start=(c == 0), stop=(c == 1), perf_mode=DR)
        return softmax_np(ps_s, LE, "pca"), ve

    def ca_tail(h, e, proj, p_ve):
        _, _, wo, _ = proj
        (p_t, r), ve = p_ve
        pts = []
        for kc in range(KC):
            pt_t = ptp.tile([128, 128], BF16, tag="pts", name="pts")
            nc.sync.dma_start_transpose(out=pt_t,
                                        in_=p_t[:, kc * 128:(kc + 1) * 128])
            pts.append(pt_t)
        ps_c = psC.tile([128, 512], F32, tag="pc", name="pc")
        for dc in range(KC):
            for kc in range(KC):
                nc.tensor.matmul(ps_c[:, dc * 128:(dc + 1) * 128],
                                 ve[kc][:, dc * 128:(dc + 1) * 128],
                                 pts[kc], start=(kc == 0),
                                 stop=(kc == KC - 1))
        ct8 = [ptp.tile([128, 2, 128], FP8, tag="ct8", name="ct8")
               for _ in range(2)]
        for c in range(2):
            eng.copy(ct8[c].rearrange("p i j -> p (i j)"),
                     ps_c[:, c * 256:(c + 1) * 256])
        ps_o = psP.tile([128, 512], F32, tag="pp", name="pp")
        for c in range(2):
            nc.tensor.matmul(ps_o, ct8[c], wo[c], start=(c == 0),
                             stop=(c == 1), perf_mode=DR)
        if h == 0:
            acc_ca[e] = accp.tile([128, D], F32, tag="acc", name="acc")
            nc.vector.scalar_tensor_tensor(out=acc_ca[e], in0=ps_o, scalar=r,
                                           in1=x1[e], op0=ALU.mult,
                                           op1=ALU.add)
        else:
            nc.vector.scalar_tensor_tensor(out=acc_ca[e], in0=ps_o, scalar=r,
                                           in1=acc_ca[e], op0=ALU.mult,
                                           op1=ALU.add)

    ff1, ff2 = {}, []

    def load_ff():
        for dc in range(KC):
            for hq in range(DF // 512):
                t = ar.tile([128, 512], BF16, tag="a", name="a")
                nc.sync.dma_start(
                    out=t, in_=w_d["ff1"][dc * 128:(dc + 1) * 128,
                                          hq * 512:(hq + 1) * 512])
                ff1[(dc, hq)] = t
        for hc in range(DF // 128):
            t = ar.tile([128, 512], BF16, tag="a", name="a")
            nc.sync.dma_start(out=t, in_=w_d["ff2"][hc * 128:(hc + 1) * 128, :])
            ff2.append(t)

    # CA h=0 K/V hoisted around the SA->CA layernorm boundary: independent
    # PE work that fills the LN/transpose latency.
    wk0 = load_w8("ca_k8", 0)
    wv0 = load_w8("ca_v8", 0)
    kv0 = [ca_kv(0, e, wk0, wv0) for e in range(BPC)]

    x1 = [layer_norm(acc_sa[e], "x") for e in range(BPC)]
    x1t8 = [xtp.tile([128, 2, T], FP8, tag="x1t", name="x1t") for _ in range(2)]
    for dc in range(KC):
        for e in range(BPC):
            tp_ps = psC.tile([128, 128], F32, tag="pc", name="pc")
            nc.tensor.transpose(tp_ps, x1[e][:, dc * 128:(dc + 1) * 128],
                                ident_f32)
            eng.copy(x1t8[dc // 2][:, dc % 2, e * 128:(e + 1) * 128], tp_ps)

    pend = []
    for h in range(H):
        if h == 0:
            wo = load_wo8("ca_o8", 0)
            qt8 = proj_dr8(load_w8("ca_q8", 0), x1t8, "caq", 0)
            proj = (wk0, wv0, wo, qt8)
        else:
            proj = ca_proj(h)
        if h == 2:
            load_ff()
        for e in range(BPC):
            p_ve = ca_scores(h, e, proj, kv=kv0[e] if h == 0 else None)
            pend.append((h, e, proj, p_ve))
            if len(pend) > 4:
                ca_tail(*pend.pop(0))
    for u in pend:
        ca_tail(*u)

    x2 = [layer_norm(acc_ca[e], "x") for e in range(BPC)]
    x2t = [xtp.tile([128, T], BF16, tag="x2t", name="x2t") for _ in range(KC)]
    for dc in range(KC):
        for e in range(BPC):
            tp_ps = psC.tile([128, 128], F32, tag="pc", name="pc")
            nc.tensor.transpose(tp_ps, x2[e][:, dc * 128:(dc + 1) * 128],
                                ident_f32)
            eng.copy(x2t[dc][:, e * 128:(e + 1) * 128], tp_ps)

    # ================= feed-forward =================

    hT = []
    for hc in range(DF // 128):
        ps = psP.tile([128, T], F32, tag="pp", name="pp")
        for dc in range(KC):
            nc.tensor.matmul(
                ps, ff1[(dc, hc // 4)][:, (hc % 4) * 128:(hc % 4 + 1) * 128],
                x2t[dc], start=(dc == 0), stop=(dc == KC - 1))
        t = htp.tile([128, T], BF16, tag="ht", name="ht")
        if hc % 2 == 0:
            nc.vector.tensor_scalar_max(t, ps, 0.0)
        else:
            nc.scalar.activation(out=t, in_=ps, func=AF.Relu)
        hT.append(t)

    for e in range(BPC):
        ps_o = psP.tile([128, 512], F32, tag="pp", name="pp")
        for hc in range(DF // 128):
            nc.tensor.matmul(ps_o, hT[hc][:, e * 128:(e + 1) * 128],
                             ff2[hc], start=(hc == 0), stop=(hc == DF // 128 - 1))
        accf = accp.tile([128, D], F32, tag="acc", name="acc")
        nc.vector.tensor_add(accf, ps_o, x2[e])
        xn = layer_norm(accf, "x")
        nc.sync.dma_start(out=out_d[e * 128:(e + 1) * 128, :], in_=xn)


def _host_prep(inputs):
    """Build the 8 per-core input maps from full inputs."""
    gi = {k: np.asarray(v) for k, v in inputs.items()}
    bf = ml_dtypes.bfloat16
    f8 = ml_dtypes.float8_e4m3

    def pack8(w):
        # [512, C] -> [c=2, p=128, i=2, C] with row = c*256 + i*128 + p
        return np.ascontiguousarray(
            w.astype(f8).reshape(2, 2, 128, -1).transpose(0, 2, 1, 3))

    def pack8_oh(w):
        # [DH, D] -> [H, 2, 128, 2, D] per-head pack8 of the rows
        return np.ascontiguousarray(
            w.astype(f8).reshape(H, 2, 2, 128, -1).transpose(0, 1, 3, 2, 4))

    wmap = {}
    for pre in ("sa", "ca"):
        for nm in "qkv":
            wmap[f"w_{pre}{nm}8"] = pack8(gi[f"{pre}_w{nm}"])
        wmap[f"w_{pre}o8"] = pack8_oh(gi[f"{pre}_wo"])
    wmap["w_ff1"] = gi["ff_w1"].astype(bf)
    wmap["w_ff2"] = gi["ff_w2"].astype(bf)

    for k, src in (("saq", "sa_bq"), ("sak", "sa_bk"), ("sav", "sa_bv"),
                   ("caq", "ca_bq"), ("cak", "ca_bk"), ("cav", "ca_bv")):
        wmap[f"bp_{k}"] = np.ascontiguousarray(
            gi[src].astype(np.float32).reshape(DH // 128, 128).T)
    # The kernel folds the softmax 1/rowsum into the residual update and
    # drops the LN gamma/beta and V/O-bias ops; exact only for the
    # structurally-fixed values this module is defined with:
    for k in ("sa_g", "ca_g", "ff_g"):
        assert np.allclose(gi[k], 1.0), k
    for k in ("sa_b", "ca_b", "ff_b", "sa_bv", "ca_bv", "ca_bo"):
        assert np.allclose(gi[k], 0.0), k

    in_maps = []
    for c in range(NCORES):
        sl = slice(c * BPC, (c + 1) * BPC)
        dec = gi["dec_inputs"][sl].astype(np.float32)          # [4,128,512]
        enc = gi["enc_outputs"][sl].astype(np.float32)         # [4,512,512]
        msk = gi["dec_self_attn_mask"][sl]                     # [4,128,128]
        m = dict(wmap)
        xTf = np.ascontiguousarray(
            dec.transpose(2, 0, 1).reshape(D, T))              # [512, T]
        m["xT8"] = np.ascontiguousarray(
            xTf.reshape(2, 2, 128, T).transpose(0, 2, 1, 3)).astype(f8)
        m["x0"] = np.ascontiguousarray(
            dec.reshape(T, D) + gi["sa_bo"].astype(np.float32)[None, :])
        m["encT8"] = np.ascontiguousarray(
            enc.transpose(0, 2, 1).reshape(BPC, 2, 2, 128, LE)
            .transpose(0, 1, 3, 2, 4)).astype(f8)
        m["maskneg"] = np.ascontiguousarray(
            np.where(msk, np.float32(-1e9), np.float32(0.0))
            .transpose(1, 0, 2).reshape(LD, T))
        in_maps.append(m)
    return in_maps


def _get_compiled(loop_n=1):
    key = f"nc{loop_n}"
    if key not in _CACHE:
        _CACHE[key] = _build(loop_n)
    return _CACHE[key]


def kernel(**inputs):
    nc = _get_compiled()
    in_maps = _host_prep(inputs)
    res = run_bass_kernel_spmd(nc, in_maps, core_ids=list(range(NCORES)))
    out = np.concatenate(
        [res.results[c]["out"].reshape(BPC, LD, D) for c in range(NCORES)],
        axis=0)
    return out.astype(np.float32)


# revision 24
# speedup vs baseline: 1.5407x; 1.5407x over previous
"""Trainium2 Bass kernel for nn_DecoderLayer (self-attn + cross-attn + FFN).

Sharding: data-parallel over batch, 4 batch elements per core x 8 cores.
Each core runs an identical (SPMD) Tile program on its own shard; no
collectives.

v2: fp8(e4m3) DoubleRow matmuls for SA Q/K/V/scores/out-proj and
CA Q/scores/ctx/out-proj (CA K/V were already fp8-DR); FFN and SA ctx
stay bf16. Softmax skips the max-subtraction (scores are tiny: |s|*scale
<~1.5). PSUM evacuations alternate DVE/ACT; gpsimd takes SBUF->SBUF
copies. Measured L2 rel err ~1.1e-2 on host sim (gate 2e-2).

Layouts (per core, T = 4*128 = 512 decoder tokens, LE = 512 enc tokens):
  xT8     [2, 128, 2, T]  fp8  dec inputs feature-major in DoubleRow
                               layout: [c, p, i, t] = x[c*256+i*128+p, t]
  x0      [T, D]  f32          dec inputs token-major (residual + sa_bo)
  encT8   [BPC, 2, 128, 2, LE] fp8 enc outputs feature-major DR layout
  maskneg [128, T] f32         -1e9 where masked, [q, e*128+k]
DR matmul operands are [128, 2, N] fp8 (contraction pairs on the middle
axis); weight tensors are host-packed into that layout (pack8).
"""

import contextlib
import os
import sys

for _p in ('/opt/trn_rl_repo', '/root/.axon_site/_ro/trn_rl_repo'):
    if os.path.isdir(_p) and _p not in sys.path:
        sys.path.append(_p)

import numpy as np
import ml_dtypes

import concourse.bass as bass
import concourse.tile as tile
import concourse.mybir as mybir
from concourse import bacc
from concourse.bass_utils import run_bass_kernel_spmd
from concourse.masks import make_identity

F32 = mybir.dt.float32
BF16 = mybir.dt.bfloat16
FP8 = mybir.dt.float8e4
DR = mybir.MatmulPerfMode.DoubleRow
AF = mybir.ActivationFunctionType
ALU = mybir.AluOpType
AX = mybir.AxisListType

B, LD, LE, D, H, R = 32, 128, 512, 512, 8, 4
DH = D * H            # 4096
DF = D * R            # 2048
NCORES = 8
BPC = B // NCORES     # 4 batch elements per core
T = BPC * LD          # 512 decoder tokens per core
KC = D // 128         # 4 contraction chunks of 128
SCALE = float(1.0 / np.sqrt(D))

_CACHE = {}


class _Eng:
    """Alternating DVE/ACT picker for PSUM->SBUF evacuation."""

    def __init__(self, nc, pat="01"):
        self.nc = nc
        self.pat = pat
        self.i = 0

    def copy(self, out, in_, bias=None):
        nc = self.nc
        self.i = (self.i + 1) % len(self.pat)
        if self.pat[self.i] == "0":
            if bias is None:
                nc.vector.tensor_copy(out=out, in_=in_)
            else:
                nc.vector.tensor_scalar_add(out, in_, bias)
        else:
            if bias is None:
                nc.scalar.copy(out, in_)
            else:
                nc.scalar.activation(out=out, in_=in_, func=AF.Identity, bias=bias)


_POOLSPEC = [
    ("const", 1, "SBUF"), ("aring", 72, "SBUF"), ("wp", 20, "SBUF"),
    ("wop", 3, "SBUF"), ("encp", 8, "SBUF"),
    ("xfp", 6, "SBUF"), ("accp", 6, "SBUF"), ("xtp", 8, "SBUF"),
    ("htp", 16, "SBUF"), ("ctp", 12, "SBUF"), ("pp", 8, "SBUF"),
    ("ptp", 16, "SBUF"), ("stp", 24, "SBUF"), ("bnp", 4, "SBUF"),
    ("psP", 2, "PSUM"), ("psS", 2, "PSUM"), ("psC", 2, "PSUM"),
    ("psT", 2, "PSUM"),
]


def _build(loop_n=1):
    nc = bacc.Bacc("TRN2", target_bir_lowering=False, debug=False,
                   num_devices=NCORES)

    def din(name, shape, dt):
        return nc.dram_tensor(name, shape, dt, kind="ExternalInput").ap()

    xT8_d = din("xT8", [2, 128, 2, T], FP8)
    x0_d = din("x0", [T, D], F32)
    encT_d = din("encT8", [BPC, 2, 128, 2, LE], FP8)
    mask_d = din("maskneg", [LD, T], F32)

    w_d = {}
    for pre in ("sa", "ca"):
        for nm in "qkv" if pre == "sa" else "qkv":
            w_d[f"{pre}_{nm}8"] = din(f"w_{pre}{nm}8", [2, 128, 2, DH], FP8)
        w_d[f"{pre}_o8"] = din(f"w_{pre}o8", [H, 2, 128, 2, D], FP8)
    w_d["ff1"] = din("w_ff1", [D, DF], BF16)
    w_d["ff2"] = din("w_ff2", [DF, D], BF16)

    bp_d = {k: din(f"bp_{k}", [128, DH // 128], F32)
            for k in ("saq", "sak", "sav", "caq", "cak", "cav")}

    out_d = nc.dram_tensor("out", [T, D], F32, kind="ExternalOutput").ap()

    with tile.TileContext(nc) as tc:
        with contextlib.ExitStack() as _st:
            pools = {}
            for _nm, _bufs, _sp in _POOLSPEC:
                pools[_nm] = _st.enter_context(
                    tc.tile_pool(name=_nm, bufs=_bufs, space=_sp))
            if loop_n > 1:
                _st.enter_context(tc.For_i(0, loop_n, 1))
            _emit(nc, tc, pools, xT8_d, x0_d, encT_d, mask_d,
                  w_d, bp_d, out_d)
    nc.compile()
    return nc


def _emit(nc, tc, pools, xT8_d, x0_d, encT_d, mask_d, w_d, bp_d, out_d):
    cpool, ar, encp, xfp = pools["const"], pools["aring"], pools["encp"], pools["xfp"]
    wpool = pools["wp"]
    accp, xtp, htp, ctp = pools["accp"], pools["xtp"], pools["htp"], pools["ctp"]
    ppool, ptp, stp, bnp = pools["pp"], pools["ptp"], pools["stp"], pools["bnp"]
    psP, psS, psC, psT = pools["psP"], pools["psS"], pools["psC"], pools["psT"]

    eng = _Eng(nc)

    # ---------------- constants ----------------
    ident_bf = cpool.tile([128, 128], BF16, tag="idb", name="idb")
    make_identity(nc, ident_bf)
    ident_f32 = cpool.tile([128, 128], F32, tag="idf", name="idf")
    make_identity(nc, ident_f32)
    eps_t = cpool.tile([128, 1], F32, tag="eps", name="eps")
    nc.vector.memset(eps_t, 1e-5)

    # ---------------- activations in ----------------
    xT8 = []
    for c in range(2):
        t = xtp.tile([128, 2, T], FP8, tag="xt8", name="xt8")
        nc.sync.dma_start(out=t, in_=xT8_d[c])
        xT8.append(t)
    mask_t = cpool.tile([128, T], F32, tag="mask", name="mask")
    nc.sync.dma_start(out=mask_t, in_=mask_d)
    bp = {}
    for k, d in bp_d.items():
        t = cpool.tile([128, DH // 128], F32, tag=f"bp_{k}", name=f"bp_{k}")
        nc.sync.dma_start(out=t, in_=d)
        bp[k] = t

    def load_w8(key, h, dmae=None):
        """[2] tiles [128, 2, 512] fp8 from dram [2, 128, 2, DH]."""
        ts = []
        for c in range(2):
            t = wpool.tile([128, 2, 512], FP8, tag="w8", name="w8")
            (dmae or nc.sync).dma_start(
                out=t, in_=w_d[key][c, :, :, h * 512:(h + 1) * 512])
            ts.append(t)
        return ts

    def load_wo8(key, h):
        """[2] tiles [128, 2, 512] fp8 from dram [H, 2, 128, 2, D]."""
        ts = []
        for c in range(2):
            t = wpool.tile([128, 2, 512], FP8, tag="w8", name="w8")
            nc.sync.dma_start(out=t, in_=w_d[key][h, c])
            ts.append(t)
        return ts

    def proj_dr8(w2, rhs2, bias_key, h, width=T):
        """DR projection -> 2 tiles [128, 2, width] fp8 (DR layout)."""
        outs = [ar.tile([128, 2, width], FP8, tag="a", name="a")
                for _ in range(2)]
        for dco in range(KC):
            ps = psP.tile([128, width], F32, tag="pp", name="pp")
            for c in range(2):
                nc.tensor.matmul(ps, w2[c][:, :, dco * 128:(dco + 1) * 128],
                                 rhs2[c], start=(c == 0), stop=(c == 1),
                                 perf_mode=DR)
            bcol = bp[bias_key][:, h * 4 + dco:h * 4 + dco + 1]
            eng.copy(outs[dco // 2][:, dco % 2, :], ps, bias=bcol)
        return outs

    def proj_bf(w2, rhs2, bias_key, h, width=T):
        """DR projection -> KC tiles [128, width] bf16 (plain layout)."""
        outs = []
        for dco in range(KC):
            ps = psP.tile([128, width], F32, tag="pp", name="pp")
            for c in range(2):
                nc.tensor.matmul(ps, w2[c][:, :, dco * 128:(dco + 1) * 128],
                                 rhs2[c], start=(c == 0), stop=(c == 1),
                                 perf_mode=DR)
            t = ar.tile([128, width], BF16, tag="a", name="a")
            bcol = bp[bias_key][:, h * 4 + dco:h * 4 + dco + 1]
            eng.copy(t, ps, bias=bcol)
            outs.append(t)
        return outs

    def softmax_np(ps_s, width, p_tag):
        """exp(scale*s) with row-sum accum; no max subtraction.

        Returns (unnormalized P tile, 1/rowsum [128,1]); the 1/rowsum is
        folded into the residual update after the output projection
        (exact given zero V/O biases, asserted host-side).
        """
        p_t = ppool.tile([128, width], BF16, tag=p_tag, name=p_tag)
        rs = stp.tile([128, 1], F32, tag="st", name="st")
        nc.scalar.activation(out=p_t, in_=ps_s, func=AF.Exp,
                             scale=SCALE, accum_out=rs)
        r = stp.tile([128, 1], F32, tag="st", name="st")
        nc.vector.reciprocal(r, rs)
        return p_t, r

    def layer_norm(acc, out_tag):
        """returns normed f32 tile (gamma=1/beta=0 asserted host-side)."""
        bn = bnp.tile([128, 6], F32, tag="bn", name="bn")
        nc.vector.bn_stats(out=bn, in_=acc)
        mv = bnp.tile([128, 2], F32, tag="mv", name="mv")
        nc.vector.bn_aggr(out=mv, in_=bn)
        std = stp.tile([128, 1], F32, tag="st", name="st")
        nc.scalar.activation(out=std, in_=mv[:, 1:2], func=AF.Sqrt,
                             bias=eps_t)
        rstd = stp.tile([128, 1], F32, tag="st", name="st")
        nc.vector.reciprocal(rstd, std)
        nb = stp.tile([128, 1], F32, tag="st", name="st")
        nc.vector.tensor_scalar(out=nb, in0=mv[:, 0:1], scalar1=rstd,
                                scalar2=-1.0, op0=ALU.mult, op1=ALU.mult)
        xn = xfp.tile([128, D], F32, tag=out_tag, name=out_tag)
        nc.scalar.activation(out=xn, in_=acc, func=AF.Identity,
                             scale=rstd, bias=nb)
        return xn

    # ================= self attention =================
    acc_sa = [None] * BPC
    x0 = []

    def sa_proj(h):
        dmae = nc.gpsimd if h == 0 else None
        wq = load_w8("sa_q8", h, dmae)
        wk = load_w8("sa_k8", h, dmae)
        wv = load_w8("sa_v8", h)
        wo = load_wo8("sa_o8", h)
        qth = proj_bf(wq, xT8, "saq", h)
        kth = proj_bf(wk, xT8, "sak", h)
        vh = []
        for e in range(BPC):
            ps = psP.tile([128, 512], F32, tag="pp", name="pp")
            for c in range(2):
                nc.tensor.matmul(ps, xT8[c][:, :, e * 128:(e + 1) * 128],
                                 wv[c], start=(c == 0), stop=(c == 1),
                                 perf_mode=DR)
            t = ar.tile([128, 512], BF16, tag="a", name="a")
            eng.copy(t, ps)
            vh.append(t)
        return qth, kth, vh, wo

    def sa_scores(h, e, proj):
        qth, kth, vh, wo = proj
        sl = slice(e * 128, (e + 1) * 128)
        ps_s = psS.tile([128, 512], F32, tag="ps", name="ps")
        ss = ps_s[:, 0:128]
        for dc in range(KC):
            nc.tensor.matmul(ss, qth[dc][:, sl], kth[dc][:, sl],
                             start=(dc == 0), stop=(dc == KC - 1))
        nc.vector.tensor_add(ss, ss, mask_t[:, sl])
        return softmax_np(ss, 128, "psa")

    def sa_tail(h, e, proj, p_r):
        _, _, vh, wo = proj
        p_t, r = p_r
        tp_ps = psT.tile([128, 128], BF16, tag="pt", name="pt")
        nc.tensor.transpose(tp_ps, p_t, ident_bf)
        pt_t = ptp.tile([128, 128], BF16, tag="pts", name="pts")
        eng.copy(pt_t, tp_ps)
        ps_c = psC.tile([128, 512], F32, tag="pc", name="pc")
        for dc in range(KC):
            nc.tensor.matmul(ps_c[:, dc * 128:(dc + 1) * 128],
                             vh[e][:, dc * 128:(dc + 1) * 128], pt_t,
                             start=True, stop=True)
        ct8 = [ptp.tile([128, 2, 128], FP8, tag="ct8", name="ct8")
               for _ in range(2)]
        for c in range(2):
            eng.copy(ct8[c].rearrange("p i j -> p (i j)"),
                     ps_c[:, c * 256:(c + 1) * 256])
        ps_o = psP.tile([128, 512], F32, tag="pp", name="pp")
        for c in range(2):
            nc.tensor.matmul(ps_o, ct8[c], wo[c], start=(c == 0),
                             stop=(c == 1), perf_mode=DR)
        if h == 0:
            t = xfp.tile([128, D], F32, tag="x", name="x")
            nc.sync.dma_start(out=t, in_=x0_d[e * 128:(e + 1) * 128, :])
            x0.append(t)
            acc_sa[e] = accp.tile([128, D], F32, tag="acc", name="acc")
            nc.vector.scalar_tensor_tensor(out=acc_sa[e], in0=ps_o, scalar=r,
                                           in1=x0[e], op0=ALU.mult,
                                           op1=ALU.add)
        else:
            nc.vector.scalar_tensor_tensor(out=acc_sa[e], in0=ps_o, scalar=r,
                                           in1=acc_sa[e], op0=ALU.mult,
                                           op1=ALU.add)

    encT = [[None] * 2 for _ in range(BPC)]

    def load_enc():
        for e in range(BPC):
            for c in range(2):
                t = encp.tile([128, 2, LE], FP8, tag="enc", name="enc")
                nc.sync.dma_start(out=t, in_=encT_d[e, c])
                encT[e][c] = t

    pend = []
    for h in range(H):
        proj = sa_proj(h)
        if h == 6:
            load_enc()
        for e in range(BPC):
            p_t = sa_scores(h, e, proj)
            pend.append((h, e, proj, p_t))
            if len(pend) > 4:
                sa_tail(*pend.pop(0))
    for u in pend:
        sa_tail(*u)

    # ================= cross attention =================
    acc_ca = [None] * BPC

    def ca_proj(h):
        wk = load_w8("ca_k8", h)
        wv = load_w8("ca_v8", h)
        wo = load_wo8("ca_o8", h)
        qt8 = proj_dr8(load_w8("ca_q8", h), x1t8, "caq", h)
        return wk, wv, wo, qt8

    def ca_kv(h, e, wk, wv):
        """per-elem K (fp8 DR layout, for DR scores) and V (bf16 token-major)."""
        kt8e = [ar.tile([128, 2, LE], FP8, tag="a", name="a") for _ in range(2)]
        for mc in range(KC):
            ps = psP.tile([128, LE], F32, tag="pp", name="pp")
            for c in range(2):
                nc.tensor.matmul(ps, wk[c][:, :, mc * 128:(mc + 1) * 128],
                                 encT[e][c], start=(c == 0), stop=(c == 1),
                                 perf_mode=DR)
            eng.copy(kt8e[mc // 2][:, mc % 2, :], ps,
                     bias=bp["cak"][:, h * 4 + mc:h * 4 + mc + 1])
        ve = []
        for tc_ in range(KC):
            ps = psP.tile([128, 512], F32, tag="pp", name="pp")
            for c in range(2):
                nc.tensor.matmul(ps, encT[e][c][:, :, tc_ * 128:(tc_ + 1) * 128],
                                 wv[c], start=(c == 0), stop=(c == 1),
                                 perf_mode=DR)
            t = ar.tile([128, 512], BF16, tag="a", name="a")
            eng.copy(t, ps)
            ve.append(t)
        return kt8e, ve

    def ca_scores(h, e, proj, kv=None):
        wk, wv, wo, qt8 = proj
        kt8e, ve = kv if kv is not None else ca_kv(h, e, wk, wv)
        sl = slice(e * 128, (e + 1) * 128)
        ps_s = psS.tile([128, LE], F32, tag="ps", name="ps")
        for c in range(2):
            nc.tensor.matmul(ps_s, qt8[c][:, :, sl], kt8e[c],
                             start=(c == 0), stop=(c == 1), perf_mode=DR)
        return softmax_np(ps_s, LE, "pca"), ve

    def ca_tail(h, e, proj, p_ve):
        _, _, wo, _ = proj
        (p_t, r), ve = p_ve
        pts = []
        for kc in range(KC):
            tp_ps = psT.tile([128, 128], BF16, tag="pt", name="pt")
            nc.tensor.transpose(tp_ps, p_t[:, kc * 128:(kc + 1) * 128],
                                ident_bf)
            pt_t = ptp.tile([128, 128], BF16, tag="pts", name="pts")
            eng.copy(pt_t, tp_ps)
            pts.append(pt_t)
        ps_c = psC.tile([128, 512], F32, tag="pc", name="pc")
        for dc in range(KC):
            for kc in range(KC):
                nc.tensor.matmul(ps_c[:, dc * 128:(dc + 1) * 128],
                                 ve[kc][:, dc * 128:(dc + 1) * 128],
                                 pts[kc], start=(kc == 0),
                                 stop=(kc == KC - 1))
        ct8 = [ptp.tile([128, 2, 128], FP8, tag="ct8", name="ct8")
               for _ in range(2)]
        for c in range(2):
            eng.copy(ct8[c].rearrange("p i j -> p (i j)"),
                     ps_c[:, c * 256:(c + 1) * 256])
        ps_o = psP.tile([128, 512], F32, tag="pp", name="pp")
        for c in range(2):
            nc.tensor.matmul(ps_o, ct8[c], wo[c], start=(c == 0),
                             stop=(c == 1), perf_mode=DR)
        if h == 0:
            acc_ca[e] = accp.tile([128, D], F32, tag="acc", name="acc")
            nc.vector.scalar_tensor_tensor(out=acc_ca[e], in0=ps_o, scalar=r,
                                           in1=x1[e], op0=ALU.mult,
                                           op1=ALU.add)
        else:
            nc.vector.scalar_tensor_tensor(out=acc_ca[e], in0=ps_o, scalar=r,
                                           in1=acc_ca[e], op0=ALU.mult,
                                           op1=ALU.add)

    ff1, ff2 = {}, []

    def load_ff():
        for dc in range(KC):
            for hq in range(DF // 512):
                t = ar.tile([128, 512], BF16, tag="a", name="a")
                nc.sync.dma_start(
                    out=t, in_=w_d["ff1"][dc * 128:(dc + 1) * 128,
                                          hq * 512:(hq + 1) * 512])
                ff1[(dc, hq)] = t
        for hc in range(DF // 128):
            t = ar.tile([128, 512], BF16, tag="a", name="a")
            nc.sync.dma_start(out=t, in_=w_d["ff2"][hc * 128:(hc + 1) * 128, :])
            ff2.append(t)

    # CA h=0 K/V hoisted around the SA->CA layernorm boundary: independent
    # PE work that fills the LN/transpose latency.
    wk0 = load_w8("ca_k8", 0)
    wv0 = load_w8("ca_v8", 0)
    kv0 = [ca_kv(0, e, wk0, wv0) for e in range(BPC)]

    x1 = [layer_norm(acc_sa[e], "x") for e in range(BPC)]
    x1t8 = [xtp.tile([128, 2, T], FP8, tag="x1t", name="x1t") for _ in range(2)]
    for dc in range(KC):
        for e in range(BPC):
            tp_ps = psC.tile([128, 128], F32, tag="pc", name="pc")
            nc.tensor.transpose(tp_ps, x1[e][:, dc * 128:(dc + 1) * 128],
                                ident_f32)
            eng.copy(x1t8[dc // 2][:, dc % 2, e * 128:(e + 1) * 128], tp_ps)

    pend = []
    for h in range(H):
        if h == 0:
            wo = load_wo8("ca_o8", 0)
            qt8 = proj_dr8(load_w8("ca_q8", 0), x1t8, "caq", 0)
            proj = (wk0, wv0, wo, qt8)
        else:
            proj = ca_proj(h)
        if h == 2:
            load_ff()
        for e in range(BPC):
            p_ve = ca_scores(h, e, proj, kv=kv0[e] if h == 0 else None)
            pend.append((h, e, proj, p_ve))
            if len(pend) > 4:
                ca_tail(*pend.pop(0))
    for u in pend:
        ca_tail(*u)

    x2 = [layer_norm(acc_ca[e], "x") for e in range(BPC)]
    x2t = [xtp.tile([128, T], BF16, tag="x2t", name="x2t") for _ in range(KC)]
    for dc in range(KC):
        for e in range(BPC):
            tp_ps = psC.tile([128, 128], F32, tag="pc", name="pc")
            nc.tensor.transpose(tp_ps, x2[e][:, dc * 128:(dc + 1) * 128],
                                ident_f32)
            eng.copy(x2t[dc][:, e * 128:(e + 1) * 128], tp_ps)

    # ================= feed-forward =================

    hT = []
    for hc in range(DF // 128):
        ps = psP.tile([128, T], F32, tag="pp", name="pp")
        for dc in range(KC):
            nc.tensor.matmul(
                ps, ff1[(dc, hc // 4)][:, (hc % 4) * 128:(hc % 4 + 1) * 128],
                x2t[dc], start=(dc == 0), stop=(dc == KC - 1))
        t = htp.tile([128, T], BF16, tag="ht", name="ht")
        if hc % 2 == 0:
            nc.vector.tensor_scalar_max(t, ps, 0.0)
        else:
            nc.scalar.activation(out=t, in_=ps, func=AF.Relu)
        hT.append(t)

    for e in range(BPC):
        ps_o = psP.tile([128, 512], F32, tag="pp", name="pp")
        for hc in range(DF // 128):
            nc.tensor.matmul(ps_o, hT[hc][:, e * 128:(e + 1) * 128],
                             ff2[hc], start=(hc == 0), stop=(hc == DF // 128 - 1))
        accf = accp.tile([128, D], F32, tag="acc", name="acc")
        nc.vector.tensor_add(accf, ps_o, x2[e])
        xn = layer_norm(accf, "x")
        nc.sync.dma_start(out=out_d[e * 128:(e + 1) * 128, :], in_=xn)


def _host_prep(inputs):
    """Build the 8 per-core input maps from full inputs."""
    gi = {k: np.asarray(v) for k, v in inputs.items()}
    bf = ml_dtypes.bfloat16
    f8 = ml_dtypes.float8_e4m3

    def pack8(w):
        # [512, C] -> [c=2, p=128, i=2, C] with row = c*256 + i*128 + p
        return np.ascontiguousarray(
            w.astype(f8).reshape(2, 2, 128, -1).transpose(0, 2, 1, 3))

    def pack8_oh(w):
        # [DH, D] -> [H, 2, 128, 2, D] per-head pack8 of the rows
        return np.ascontiguousarray(
            w.astype(f8).reshape(H, 2, 2, 128, -1).transpose(0, 1, 3, 2, 4))

    wmap = {}
    for pre in ("sa", "ca"):
        for nm in "qkv":
            wmap[f"w_{pre}{nm}8"] = pack8(gi[f"{pre}_w{nm}"])
        wmap[f"w_{pre}o8"] = pack8_oh(gi[f"{pre}_wo"])
    wmap["w_ff1"] = gi["ff_w1"].astype(bf)
    wmap["w_ff2"] = gi["ff_w2"].astype(bf)

    for k, src in (("saq", "sa_bq"), ("sak", "sa_bk"), ("sav", "sa_bv"),
                   ("caq", "ca_bq"), ("cak", "ca_bk"), ("cav", "ca_bv")):
        wmap[f"bp_{k}"] = np.ascontiguousarray(
            gi[src].astype(np.float32).reshape(DH // 128, 128).T)
    # The kernel folds the softmax 1/rowsum into the residual update and
    # drops the LN gamma/beta and V/O-bias ops; exact only for the
    # structurally-fixed values this module is defined with:
    for k in ("sa_g", "ca_g", "ff_g"):
        assert np.allclose(gi[k], 1.0), k
    for k in ("sa_b", "ca_b", "ff_b", "sa_bv", "ca_bv", "ca_bo"):
        assert np.allclose(gi[k], 0.0), k

    in_maps = []
    for c in range(NCORES):
        sl = slice(c * BPC, (c + 1) * BPC)
        dec = gi["dec_inputs"][sl].astype(np.float32)          # [4,128,512]
        enc = gi["enc_outputs"][sl].astype(np.float32)         # [4,512,512]
        msk = gi["dec_self_attn_mask"][sl]                     # [4,128,128]
        m = dict(wmap)
        xTf = np.ascontiguousarray(
            dec.transpose(2, 0, 1).reshape(D, T))              # [512, T]
        m["xT8"] = np.ascontiguousarray(
            xTf.reshape(2, 2, 128, T).transpose(0, 2, 1, 3)).astype(f8)
        m["x0"] = np.ascontiguousarray(
            dec.reshape(T, D) + gi["sa_bo"].astype(np.float32)[None, :])
        m["encT8"] = np.ascontiguousarray(
            enc.transpose(0, 2, 1).reshape(BPC, 2, 2, 128, LE)
            .transpose(0, 1, 3, 2, 4)).astype(f8)
        m["maskneg"] = np.ascontiguousarray(
            np.where(msk, np.float32(-1e9), np.float32(0.0))
            .transpose(1, 0, 2).reshape(LD, T))
        in_maps.append(m)
    return in_maps


def _get_compiled(loop_n=1):
    key = f"nc{loop_n}"
    if key not in _CACHE:
        _CACHE[key] = _build(loop_n)
    return _CACHE[key]


def kernel(**inputs):
    nc = _get_compiled()
    in_maps = _host_prep(inputs)
    res = run_bass_kernel_spmd(nc, in_maps, core_ids=list(range(NCORES)))
    out = np.concatenate(
        [res.results[c]["out"].reshape(BPC, LD, D) for c in range(NCORES)],
        axis=0)
    return out.astype(np.float32)
